# revision 4
# baseline (speedup 1.0000x reference)
"""v4: v3 + bf16 scores w/ exact causal shrink, diag-first PV accumulation
order, feeder drain before norms, chunked x DMA, bf16 output, rebalanced
PSUM (pp2/pss3/psy3), out-proj copy on Pool.

Structure per core (core = 2*b + head-group g, 8 heads x 64 dims):
  upfront: load wk, x(tg0) -> K(0); wq -> Q(0); wv -> V(0)
  for tg in 0..3:
      B(tg): attention over t-group tg with next t-group's K/Q/V projection
             groups drained between head-pairs (before the norms, so PE is
             fed while DVE/Pool normalize).
      out-proj(tg) deferred; B(3) drains op(0..2), then op(3).
Scores bf16 (kt/qt bf16), exact causal starts (ts == toff).
PV per head-pair accumulates diagonal blocks first, a full block last, so
the final psy write isn't behind an exp->mask chain.
Denominator: ones-column row of psy -> reciprocal (DVE) -> Pool broadcast.
"""

import sys

if "/opt/trn_rl_repo" not in sys.path:
    sys.path.insert(0, "/opt/trn_rl_repo")

import ml_dtypes
import numpy as np

import concourse.bacc as bacc
import concourse.mybir as mybir
from concourse.tile import TileContext
from concourse.bass_utils import run_bass_kernel_spmd

B, T, C = 4, 2048, 1024
H_LOC = 8
D = 64
DL = H_LOC * D
P = 128
NF = 512
N_TG = T // NF
N_CS = C // P
SCALE = 1.0 / 8.0

F32 = mybir.dt.float32
F32R = mybir.dt.float32r
BF16 = mybir.dt.bfloat16
EXP = mybir.ActivationFunctionType.Exp


def build_nc(pp_bufs=2, pss_bufs=3, ex_bufs=6, psy_bufs=3,
             alloc_mode="stack", repeat=1, interleave=True,
             scores_f32r=False, out_bf16=True, diag_first=True,
             drain_before_norm=True, chunk_x=True, copy_pool=True,
             bcast_pool=True, fuse_exp=True, warmup_mm=24, tick=False):
    if fuse_exp:
        # ps2 tiles are 2 banks each: 2*2 + pp 2 + psy 2 = 8 banks
        pss_bufs, psy_bufs = 2, 2
    nc = bacc.Bacc("TRN2", target_bir_lowering=False, debug=False, num_devices=8)

    qk_dt = F32R if scores_f32r else BF16
    out_dt = BF16 if out_bf16 else F32

    tick_t = (
        nc.dram_tensor("tick", [1, 1], F32, kind="ExternalInput")
        if tick else None
    )
    xT = nc.dram_tensor("xT", [C, T], BF16, kind="ExternalInput")
    wq = nc.dram_tensor("wq", [C, DL], BF16, kind="ExternalInput")
    wk = nc.dram_tensor("wk", [C, DL], BF16, kind="ExternalInput")
    wv = nc.dram_tensor("wv", [C, DL], BF16, kind="ExternalInput")
    wp = nc.dram_tensor("wp", [DL, C], BF16, kind="ExternalInput")
    bq = nc.dram_tensor("bq", [P, DL // P], F32, kind="ExternalInput")
    bk = nc.dram_tensor("bk", [P, DL // P], F32, kind="ExternalInput")
    bv = nc.dram_tensor("bv", [P, DL], BF16, kind="ExternalInput")
    ones_in = nc.dram_tensor("ones", [1, D], F32R, kind="ExternalInput")
    outT = nc.dram_tensor("outT", [C, T], out_dt, kind="ExternalOutput")

    with TileContext(nc, pool_alloc_mode=alloc_mode) as tc:
        with (
            tc.tile_pool(name="persist", bufs=1) as persist,
            tc.tile_pool(name="wpool", bufs=1) as wpool,
            tc.tile_pool(name="attp", bufs=4) as attp,
            tc.tile_pool(name="ocpp", bufs=3) as ocpp,
            tc.tile_pool(name="att1", bufs=1) as att1,
            tc.tile_pool(name="att2", bufs=2) as att2,
            tc.tile_pool(name="xpool", bufs=1) as xpool,
            tc.tile_pool(name="psum", bufs=2, space="PSUM") as psum,
        ):
            if tick_t is not None:
                tick_sb = persist.tile([1, 1], F32, tag="tick")
                nc.sync.dma_start(out=tick_sb[:], in_=tick_t[:])

            def emit(rep):
                kt_g = [persist.tile([P, DL // P, NF], qk_dt, tag=f"kt{g}",
                                     name=f"kt{g}_{rep}")
                        for g in range(N_TG)]
                qt_g = [persist.tile([P, DL // P, NF], qk_dt, tag=f"qt{g}",
                                     name=f"qt{g}_{rep}")
                        for g in range(N_TG)]
                va_g = [persist.tile([P, 4, H_LOC, D + 1], BF16, tag=f"va{g}",
                                     name=f"va{g}_{rep}")
                        for g in range(N_TG)]
                bq_c = persist.tile([P, DL // P], F32, tag="bq")
                bk_c = persist.tile([P, DL // P], F32, tag="bk")
                bv_b = persist.tile([P, DL], BF16, tag="bv")
                ones = persist.tile([P, D], F32R, tag="ones")
                ones1 = persist.tile([1, D], F32R, tag="ones1")
                dmask = persist.tile([P, 2 * P], BF16, tag="dmask")
                nc.sync.dma_start(out=ones1[:], in_=ones_in[:])

                nc.sync.dma_start(out=bq_c[:], in_=bq[:])
                nc.sync.dma_start(out=bk_c[:], in_=bk[:])
                nc.sync.dma_start(out=bv_b[:], in_=bv[:])
                nc.sync.dma_start(out=ones[D : D + 1, :], in_=ones_in[:])
                # dmask[p, j] = 1 if j - 128 >= p else 0  (diag window at 128)
                nc.vector.memset(dmask[:], 1.0)
                nc.gpsimd.affine_select(
                    out=dmask[:],
                    in_=dmask[:],
                    compare_op=mybir.AluOpType.is_ge,
                    fill=0.0,
                    base=-P,
                    channel_multiplier=-1,
                    pattern=[[1, 2 * P]],
                )
                for g in range(N_TG):
                    nc.vector.memset(va_g[g][:, :, :, D : D + 1], 1.0)
                if bcast_pool and rep == 0:
                    # partition_broadcast lives in the gpsimd `attn` library;
                    # load it after the (native) affine_select mask init
                    from concourse import library_config

                    nc.gpsimd.load_library(library_config.attn)

                # -------- weights + x (chunked by t-group, first-need first)
                wk_sb = wpool.tile([P, N_CS, DL], BF16, tag="wk")
                wk_re = wk.ap().rearrange("(s p) d -> p s d", p=P)
                if chunk_x:
                    xt_c = {
                        (cs, g): xpool.tile([P, NF], BF16, tag=f"x{cs}_{g}",
                                            name=f"x{cs}_{g}_{rep}")
                        for cs in range(N_CS)
                        for g in range(N_TG)
                    }

                    def x_slice(cs, g):
                        return xt_c[(cs, g)][:]

                    def dma_x(g):
                        for cs in range(N_CS):
                            nc.sync.dma_start(
                                out=xt_c[(cs, g)][:],
                                in_=xT.ap()[
                                    cs * P : (cs + 1) * P,
                                    g * NF : (g + 1) * NF,
                                ],
                            )

                    # interleave wk chunks with x(tg0) chunks so the first
                    # k_group's accumulation starts as soon as chunk 0 lands
                    for cs in range(N_CS):
                        nc.sync.dma_start(
                            out=wk_sb[:, cs, :], in_=wk_re[:, cs, :]
                        )
                        nc.sync.dma_start(
                            out=xt_c[(cs, 0)][:],
                            in_=xT.ap()[cs * P : (cs + 1) * P, 0:NF],
                        )
                else:
                    nc.sync.dma_start(out=wk_sb[:], in_=wk_re)
                if not chunk_x:
                    xt_f = [xpool.tile([P, T], BF16, tag=f"x{cs}",
                                       name=f"x{cs}_{rep}")
                            for cs in range(N_CS)]
                    for cs in range(N_CS):
                        nc.sync.dma_start(
                            out=xt_f[cs][:], in_=xT.ap()[cs * P : (cs + 1) * P, :]
                        )

                    def x_slice(cs, g):
                        return xt_f[cs][:, g * NF : (g + 1) * NF]

                wq_sb = wpool.tile([P, N_CS, DL], BF16, tag="wq")
                nc.sync.dma_start(
                    out=wq_sb[:], in_=wq.ap().rearrange("(s p) d -> p s d", p=P)
                )
                wv_sb = wpool.tile([P, N_CS, DL], BF16, tag="wv")
                nc.sync.dma_start(
                    out=wv_sb[:], in_=wv.ap().rearrange("(s p) d -> p s d", p=P)
                )
                if chunk_x:
                    for g in range(1, N_TG):
                        dma_x(g)
                wp_sb = wpool.tile([P, DL // P, C], BF16, tag="wp")
                nc.sync.dma_start(
                    out=wp_sb[:], in_=wp.ap().rearrange("(s p) c -> p s c", p=P)
                )

                # dummy matmuls on the (memset-initialized) dmask tile keep
                # the PE busy during the initial DMA wait so it is at full
                # p-state when the first projection lands
                for wi in range(warmup_mm if rep == 0 else 0):
                    wps = psum.tile([P, 2 * P], F32, tag="pp",
                                    name=f"warm{wi}", bufs=pp_bufs)
                    nc.tensor.matmul(
                        wps[:],
                        dmask[:, 0:P],
                        dmask[:],
                        start=True,
                        stop=True,
                    )

                def k_group(g, dt_i):
                    ps = psum.tile([P, NF], F32, tag="pp", bufs=pp_bufs)
                    for cs in range(N_CS):
                        nc.tensor.matmul(
                            ps[:],
                            wk_sb[:, cs, dt_i * P : (dt_i + 1) * P],
                            x_slice(cs, g),
                            start=(cs == 0),
                            stop=(cs == N_CS - 1),
                        )
                    nc.vector.tensor_scalar_add(
                        kt_g[g][:, dt_i, :], ps[:], bk_c[:, dt_i : dt_i + 1]
                    )

                def q_group(g, dt_i):
                    ps = psum.tile([P, NF], F32, tag="pp", bufs=pp_bufs)
                    for cs in range(N_CS):
                        nc.tensor.matmul(
                            ps[:],
                            wq_sb[:, cs, dt_i * P : (dt_i + 1) * P],
                            x_slice(cs, g),
                            start=(cs == 0),
                            stop=(cs == N_CS - 1),
                        )
                    nc.vector.tensor_scalar_add(
                        qt_g[g][:, dt_i, :], ps[:], bq_c[:, dt_i : dt_i + 1]
                    )

                def v_group(st):
                    ps = psum.tile([P, NF], F32, tag="pp", bufs=pp_bufs)
                    for cs in range(N_CS):
                        nc.tensor.matmul(
                            ps[:],
                            x_slice(cs, st // 4)[
                                :, (st % 4) * P : (st % 4 + 1) * P
                            ],
                            wv_sb[:, cs, :],
                            start=(cs == 0),
                            stop=(cs == N_CS - 1),
                        )
                    nc.vector.tensor_add(
                        va_g[st // 4][:, st % 4, :, 0:D],
                        ps[:].rearrange("p (h d) -> p h d", d=D),
                        bv_b[:].rearrange("p (h d) -> p h d", d=D),
                    )

                def a_groups(g):
                    for dt_i in range(DL // P):
                        yield lambda dt_i=dt_i: k_group(g, dt_i)
                    for dt_i in range(DL // P):
                        yield lambda dt_i=dt_i: q_group(g, dt_i)
                    for j in range(4):
                        yield lambda j=j: v_group(4 * g + j)

                ytn_g = {}

                def op_group(tg, ct):
                    ytn = ytn_g[tg]
                    last = tg == N_TG - 1
                    if last and ct % 2:
                        # attention is over by op(N_TG-1): borrow the idle
                        # score-psum banks so more ct chains run in flight
                        pso = psum.tile([P, NF], F32, tag="pss",
                                        name="pso_s", bufs=pss_bufs)
                    else:
                        pso = psum.tile([P, NF], F32, tag="pp", bufs=pp_bufs)
                    # for the last t-group accumulate the late-finishing
                    # head-pair (pair 0, processed last there) at the end,
                    # so the js matmuls of already-normalized pairs can fill
                    # the final norm-chain latency
                    js_order = [1, 2, 3, 0] if last else [0, 1, 2, 3]
                    for i, js in enumerate(js_order):
                        nc.tensor.matmul(
                            pso[:],
                            wp_sb[:, js, ct * P : (ct + 1) * P],
                            ytn[:, js, :],
                            start=(i == 0),
                            stop=(i == DL // P - 1),
                        )
                    ocp = ocpp.tile([P, NF], out_dt, tag="ocp")
                    # Pool/GpSimd has no PSUM port; only DVE and ACT can
                    # read pso. ACT is saturated with exps until the last
                    # t-group's out-proj, where it helps drain the tail.
                    if last and copy_pool and ct % 2 == 1:
                        nc.scalar.copy(ocp[:], pso[:])
                    else:
                        nc.vector.tensor_copy(ocp[:], pso[:])
                    nc.sync.dma_start(
                        out=outT.ap()[
                            ct * P : (ct + 1) * P, tg * NF : (tg + 1) * NF
                        ],
                        in_=ocp[:],
                    )

                def op_groups(tg):
                    for ct in range(C // P):
                        yield lambda ct=ct: op_group(tg, ct)

                def emit_att(tg, feeder):
                    n_s = 4 * (tg + 1)
                    qt = qt_g[tg]
                    ytn = att2.tile([P, DL // P, NF], BF16, tag=f"ytn{tg}",
                                    name=f"ytn{tg}_{rep}", bufs=1)
                    ytn_g[tg] = ytn
                    n_feed = len(feeder)
                    fed = 0
                    # accumulation order per head-pair: diagonal blocks first
                    # (their exp->mask chains pipeline behind full-block PE
                    # work), a full block last so the final psy write has no
                    # dependency tail. First block must have toff == 0.
                    if diag_first and tg > 0:
                        si_order = list(range(4 * tg, n_s)) + list(range(4 * tg))
                    else:
                        si_order = list(range(n_s))
                    # last t-group: rotate so pair 0 is processed last --
                    # op(N_TG-1) accumulates it last (js_order) and its norm
                    # chain is the only unfillable tail
                    hp_order = ([1, 2, 3, 0] if tg == N_TG - 1
                                else list(range(H_LOC // 2)))
                    for pos, hp in enumerate(hp_order):
                        pair = (2 * hp, 2 * hp + 1)
                        psy = {
                            h: psum.tile([D + 1, NF], F32, tag="psy",
                                         name=f"psy{h}_t{tg}_{rep}",
                                         bufs=psy_bufs)
                            for h in pair
                        }

                        def flush_pv(si, exs, toff, start, stop):
                            for h in pair:
                                nc.tensor.matmul(
                                    psy[h][:, toff:] if toff else psy[h],
                                    va_g[si // 4][:, si % 4, h, :],
                                    exs[h][:, toff:],
                                    start=start,
                                    stop=stop,
                                )

                        pend = None  # delayed pv args
                        for idx, si in enumerate(si_order):
                            toff = max(0, (si - 4 * tg) * P)
                            ts = min(toff, NF - 256) if scores_f32r else toff
                            if fuse_exp:
                                ps2 = psum.tile([P, 2, NF], F32, tag="pss",
                                                name="ps2", bufs=pss_bufs)
                                pss = {h: ps2[:, i, :]
                                       for i, h in enumerate(pair)}
                            else:
                                pss = {}
                                for h in pair:
                                    pss[h] = psum.tile([P, NF], F32,
                                                       tag="pss", name="pss",
                                                       bufs=pss_bufs)
                            for h in pair:
                                rlo = D * (h % 2)
                                hs = h // 2
                                nc.tensor.matmul(
                                    pss[h][:, ts:],
                                    kt_g[si // 4][
                                        rlo : rlo + D, hs,
                                        (si % 4) * P : (si % 4 + 1) * P
                                    ],
                                    qt[rlo : rlo + D, hs, ts:],
                                    start=True,
                                    stop=True,
                                )
                            exs = {}
                            if fuse_exp:
                                ex2 = attp.tile([P, 2, NF], BF16, tag="ex",
                                                bufs=ex_bufs)
                                nc.scalar.activation(
                                    ex2[:, :, ts:], ps2[:, :, ts:], EXP,
                                    scale=SCALE
                                )
                                for i, h in enumerate(pair):
                                    if si >= 4 * tg:
                                        nc.vector.tensor_mul(
                                            ex2[:, i, ts : toff + P],
                                            ex2[:, i, ts : toff + P],
                                            dmask[:, P + ts - toff : 2 * P],
                                        )
                                    exs[h] = ex2[:, i, :]
                            else:
                                for h in pair:
                                    ex = attp.tile([P, NF], BF16, tag="ex",
                                                   bufs=ex_bufs)
                                    nc.scalar.activation(
                                        ex[:, ts:], pss[h][:, ts:], EXP,
                                        scale=SCALE
                                    )
                                    if si >= 4 * tg:  # diagonal: zero s > t
                                        nc.vector.tensor_mul(
                                            ex[:, ts : toff + P],
                                            ex[:, ts : toff + P],
                                            dmask[:, P + ts - toff : 2 * P],
                                        )
                                    exs[h] = ex
                            if pend is not None:
                                flush_pv(*pend)
                            if tg == N_TG - 1:
                                # spread the op-feeder through the si loop:
                                # queue position stops the PE front-running
                                # it all before the final dependency tail
                                prog = (pos * n_s + idx + 1) / (4.0 * n_s)
                                w_si = min(n_feed, int(n_feed * prog))
                                while fed < w_si:
                                    feeder[fed]()
                                    fed += 1
                            pend = (si, exs, toff, idx == 0, idx == n_s - 1)
                        flush_pv(*pend)

                        def _norm_pair(fast=False):
                            # phase-split across the two heads: both recips
                            # (DVE), both broadcasts (Pool/PE), both muls
                            # (DVE) overlap engine-wise. fast=True uses a PE
                            # matmul for the partition broadcast -- shorter
                            # chain and keeps the PE warm on the final pair.
                            recs = {}
                            for h in pair:
                                rec1 = att1.tile([1, NF],
                                                 F32R if fast else F32,
                                                 tag=f"dt{h % 2}",
                                                 name=f"dt{h % 2}")
                                if fast:
                                    # f32r out is fine here: the reciprocal
                                    # feeds a broadcast matmul against exact
                                    # ones; only ~2^-10 relative rounding
                                    with nc.allow_low_precision(
                                        "f32r reciprocal for broadcast"
                                    ):
                                        nc.vector.reciprocal(
                                            rec1[:], psy[h][D : D + 1, :]
                                        )
                                else:
                                    nc.vector.reciprocal(
                                        rec1[:], psy[h][D : D + 1, :]
                                    )
                                if fast:
                                    pbc = psum.tile([D, NF], F32, tag="pss",
                                                    name="pbc",
                                                    bufs=pss_bufs)
                                    nc.tensor.matmul(
                                        pbc[:],
                                        ones1[:],
                                        rec1[:],
                                        start=True,
                                        stop=True,
                                    )
                                    recs[h] = pbc
                                else:
                                    rec = att1.tile([D, NF], F32,
                                                    tag=f"rec{h % 2}",
                                                    name=f"rec{h % 2}")
                                    nc.gpsimd.partition_broadcast(
                                        rec[:], rec1[0:1, :], channels=D
                                    )
                                    recs[h] = rec
                            for h in pair:
                                hs = h // 2
                                if h % 2 == 0:
                                    nc.vector.tensor_mul(
                                        ytn[0:D, hs, :], psy[h][0:D, :],
                                        recs[h][:]
                                    )
                                else:
                                    tmp = att1.tile([D, NF], BF16, tag="tm")
                                    nc.vector.tensor_mul(
                                        tmp[:], psy[h][0:D, :], recs[h][:]
                                    )
                                    nc.sync.dma_start(
                                        out=ytn[D:P, hs, :], in_=tmp[:]
                                    )

                        # feeder distribution: early B-blocks front-load (the
                        # following block's first scores need the projections
                        # finished); the last block back-loads so plenty of
                        # out-proj PE work is queued during the final norms.
                        if tg == N_TG - 1:
                            frac = (0.167, 0.333, 0.5, 0.75)
                        else:
                            frac = (0.5, 0.75, 1.0, 1.0)
                        want = min(n_feed, int(n_feed * frac[pos] + 0.999))
                        last_pos = pos == H_LOC // 2 - 1
                        if drain_before_norm:
                            while fed < want:
                                feeder[fed]()
                                fed += 1
                            _norm_pair()
                            if last_pos:
                                while fed < n_feed:
                                    feeder[fed]()
                                    fed += 1
                        else:
                            _norm_pair()
                            while fed < want:
                                feeder[fed]()
                                fed += 1

                # -------- schedule --------
                for dt_i in range(DL // P):
                    k_group(0, dt_i)
                for dt_i in range(DL // P):
                    q_group(0, dt_i)
                for j in range(4):
                    v_group(j)
                for tg in range(N_TG):
                    if tg + 1 < N_TG:
                        feeder = list(a_groups(tg + 1))
                    else:
                        # B3 has no next projections; feed it the deferred
                        # out-projections of t-groups 0..2
                        feeder = [
                            th for t in range(N_TG - 1) for th in op_groups(t)
                        ]
                    if not interleave:
                        for th in feeder:
                            th()
                        feeder = []
                    emit_att(tg, feeder)
                for th in op_groups(N_TG - 1):
                    th()

            emit(0)
            for rep in range(1, repeat):
                emit(rep)

    nc.compile()
    return nc


def _prep_inputs(x, Wq, bq, Wk, bk, Wv, bv, Wp):
    bf = ml_dtypes.bfloat16
    in_maps = []
    for b in range(B):
        xt = np.ascontiguousarray(x[b].T).astype(bf)
        for g in range(2):
            sl = slice(g * DL, (g + 1) * DL)
            in_maps.append(
                {
                    "xT": xt,
                    "wq": np.ascontiguousarray(Wq[:, sl]).astype(bf),
                    "wk": np.ascontiguousarray(Wk[:, sl]).astype(bf),
                    "wv": np.ascontiguousarray(Wv[:, sl]).astype(bf),
                    "wp": np.ascontiguousarray(Wp[sl, :]).astype(bf),
                    "bq": np.ascontiguousarray(bq[sl].reshape(DL // P, P).T),
                    "bk": np.ascontiguousarray(bk[sl].reshape(DL // P, P).T),
                    "bv": np.ascontiguousarray(
                        np.broadcast_to(bv[sl], (P, DL))
                    ).astype(bf),
                    "ones": np.ones((1, D), np.float32),
                }
            )
    return in_maps


def kernel(x, Wq, bq, Wk, bk, Wv, bv, Wp, bp):
    x = np.asarray(x, np.float32)
    Wq, Wk, Wv, Wp = (np.asarray(a, np.float32) for a in (Wq, Wk, Wv, Wp))
    bq, bk, bv, bp = (np.asarray(a, np.float32) for a in (bq, bk, bv, bp))

    nc = build_nc()
    in_maps = _prep_inputs(x, Wq, bq, Wk, bk, Wv, bv, Wp)
    res = run_bass_kernel_spmd(nc, in_maps, core_ids=list(range(8)))

    out = np.empty((B, T, C), np.float32)
    for b in range(B):
        acc = (
            res.results[2 * b]["outT"].astype(np.float32)
            + res.results[2 * b + 1]["outT"].astype(np.float32)
        )
        out[b] = acc.T + bp
    return out
